# revision 1
# baseline (speedup 1.0000x reference)
"""Hamilton-Adams demosaic kernel for Trainium2 (8 NeuronCores, data-parallel over batch).

Input:  x [8, 4, 768, 768] f32  (Bayer quarter-res planes: P00=R, P01=Gr, P10=Gb, P11=B)
Output: [8, 3, 1536, 1536] f32  (R, G, B full-res)

Phase-domain computation in fp16: every output colour plane is assembled from its 4
phase sub-planes, so no mask multiplies are needed.  Inputs are cast f32->f16 during
the HBM->SBUF DMA (SWDGE); all stencil math runs in fp16 (DVE 16-bit 2x mode), and
the final assembly writes into f32 out tiles (stride-2 phase interleave).

Row-shifted plane copies are produced by SBUF->SBUF DMA shifts of the four main
plane tiles (no HBM re-reads).  Comparison chains are rescaled by 1/2 so that the
scalar engine's free affine (|s*x|) absorbs most constant multiplies.

Strips: 7 strips of 128 quarter-rows advancing by 124; lanes 2..125 of each strip are
shipped.  Plane tiles are [128, 770] with one halo column each side.
"""
import sys
sys.path.insert(0, '/opt/trn_rl_repo')

from contextlib import ExitStack

import numpy as np

import concourse.bass as bass
import concourse.bacc as bacc
import concourse.tile as tile
from concourse import mybir
from concourse.bass_utils import run_bass_kernel_spmd

F = mybir.dt.float32
H16 = mybir.dt.float16
U16 = mybir.dt.uint16
AL = mybir.AluOpType
AF = mybir.ActivationFunctionType

H = 768
PW = 770
NCORES = 8
STRIPS = [0, 124, 248, 372, 496, 620, 644]


BAND_ORDER = ["BA", "BB", "BC3", "BC5", "BA2", "BC5b", "BE", "BF", "BI"]


def _bands_np():
    """lhsT band matrices W[k, m] = weight of input row k toward output row m."""
    E = lambda k=0: np.eye(128, k=k, dtype=np.float32)
    B = {
        "BA": 0.5 * E(1) + 0.5 * E(0),            # 0.5*(in[m-1]+in[m])
        "BB": -0.25 * E(1) + 0.5 * E(0) - 0.25 * E(-1),  # -0.25*c3
        "BC3": 0.5 * E(1) - 1.0 * E(0) + 0.5 * E(-1),    # c3/2
        "BC5": 0.5 * E(1) - 0.5 * E(0),           # (in[m-1]-in[m])/2
        "BA2": 0.5 * E(0) + 0.5 * E(-1),          # 0.5*(in[m]+in[m+1])
        "BC5b": 0.5 * E(0) - 0.5 * E(-1),         # (in[m]-in[m+1])/2
        "BE": -0.25 * E(0) - 0.25 * E(-1),        # -0.25*(in[m]+in[m+1])
        "BF": -0.25 * E(1) - 0.25 * E(0),         # -0.25*(in[m-1]+in[m])
        "BI": 0.5 * E(0),                         # 0.5*in[m]
    }
    return np.concatenate([B[n] for n in BAND_ORDER], axis=1).astype(np.float16)


class S:
    """Tiles + helpers for one strip."""

    def __init__(self, nc, pools, r0, first, last):
        self.nc = nc
        self.r0, self.first, self.last = r0, first, last
        pl, gr, tmp, out, psp = pools
        mk = lambda n: pl.tile([128, PW], H16, tag=n, name=n)
        self.p00, self.p01, self.p10, self.p11 = mk("p00"), mk("p01"), mk("p10"), mk("p11")
        # u = shifted up (lane p holds row+1), d = shifted down (row-1)
        self.p00u, self.p11d = mk("p00u"), mk("p11d")
        gk = lambda n: gr.tile([128, PW], H16, tag=n, name=n)
        self.g00, self.g11, self.g00u, self.g11d = gk("g00"), gk("g11"), gk("g00u"), gk("g11d")
        if last:
            self.p00uz = pl.tile([128, PW], H16, tag="pz", name="p00uz")
        if first:
            self.p11dz = pl.tile([128, PW], H16, tag="pz", name="p11dz")
        ok = lambda n: out.tile([128, 3072], H16, tag=n, name=n)
        self.out_r, self.out_g, self.out_b = ok("out_r"), ok("out_g"), ok("out_b")
        self._tmp = tmp
        self._psp = psp

    def tmp(self):
        return self._tmp.tile([128, H], H16, tag="tmp", name="tmp")

    def msk(self):
        return self._tmp.tile([128, H], U16, tag="msk", name="msk")

    def ps(self, tag):
        # [128, 1024] f32 = exactly 2 PSUM banks; only cols 0:768 are used
        return self._psp.tile([128, 1024], F, tag=tag, name=tag)

    @staticmethod
    def V(t, dj=0):  # interior view of padded tile, column shift dj
        return t[:, 1 + dj : 1 + dj + H]

    @staticmethod
    def OSL(t, a, b):  # strided output slice for phase (a, b), full partitions
        base = a * 1536 + b
        return t[:, base : min(base + 1536, 3072) : 2]


def _mm(nc, ps, band, rhs, start, stop):
    """Apply one vertical band filter to a plane tile's interior (2 matmuls, N<=512)."""
    nc.tensor.matmul(ps[:, 0:512], band, rhs[:, 1:513], start=start, stop=stop)
    nc.tensor.matmul(ps[:, 512:768], band, rhs[:, 513:769], start=start, stop=stop)


def _green_phase(s, cp, hp, ho, vp, bands, bA, bC5, gdense):
    """Green at phase sites.  cp: centre plane (horiz taps), hp: horizontal
    neighbour plane at col offsets ho, vp: vertical neighbour plane.
    Vertical filters run on the tensor engine: ps_a2 = 0.5*a2 (bands bA@vp + BB@cp),
    ps_c3 = c3/2, ps_c5 = c5/2.  Comparisons at half scale (sign unchanged)."""
    nc, V = s.nc, s.V
    B = lambda n: bands[:, BAND_ORDER.index(n) * 128 : BAND_ORDER.index(n) * 128 + 128]
    ps_a2 = s.ps("psA")
    _mm(nc, ps_a2, B(bA), vp, True, False)
    _mm(nc, ps_a2, B("BB"), cp, False, True)
    ps_c3 = s.ps("psB"); _mm(nc, ps_c3, B("BC3"), cp, True, True)
    ps_c5 = s.ps("psC"); _mm(nc, ps_c5, B(bC5), vp, True, True)
    ah = s.tmp();  nc.scalar.copy(ah[:], ps_a2[:, 0:768])  # free psA quickly
    sh = s.tmp();  nc.vector.tensor_tensor(sh[:], V(cp, -1), V(cp, +1), AL.add)
    ch2 = s.tmp(); nc.vector.scalar_tensor_tensor(ch2[:], sh[:], 0.5, V(cp), AL.mult, AL.subtract)  # c2/2
    c0s = s.tmp(); nc.vector.tensor_tensor(c0s[:], V(hp, ho[0]), V(hp, ho[1]), AL.add)
    b2 = s.tmp();  nc.vector.tensor_tensor(b2[:], c0s[:], ch2[:], AL.subtract)
    c4 = s.tmp();  nc.gpsimd.tensor_tensor(c4[:], V(hp, ho[0]), V(hp, ho[1]), AL.subtract)
    q2 = s.tmp();  nc.scalar.activation(q2[:], ch2[:], AF.Abs)
    q3 = s.tmp();  nc.scalar.activation(q3[:], ps_c3[:, 0:768], AF.Abs)
    q4 = s.tmp();  nc.scalar.activation(q4[:], c4[:], AF.Abs, scale=0.5)
    q5 = s.tmp();  nc.scalar.activation(q5[:], ps_c5[:, 0:768], AF.Abs)
    clh = s.tmp(); nc.vector.tensor_tensor(clh[:], q4[:], q2[:], AL.add)
    clv = s.tmp(); nc.gpsimd.tensor_tensor(clv[:], q5[:], q3[:], AL.add)
    d = s.tmp();   nc.vector.tensor_tensor(d[:], clh[:], clv[:], AL.subtract)
    m = s.msk();   nc.vector.tensor_scalar(m[:], d[:], 0.0, None, AL.is_gt)
    nc.vector.tensor_scalar(V(gdense), b2[:], 0.5, None, AL.mult)
    nc.vector.copy_predicated(V(gdense), m[:], ah[:])


def _hv_vert_banded(s, bands, bx, xplane, bg, gplane, addp, out_t, ab):
    """Vertical hv-field fully on the tensor engine:
    out = bx@xplane + bg@gplane + BI@addp (already includes the final 0.5)."""
    nc = s.nc
    B = lambda n: bands[:, BAND_ORDER.index(n) * 128 : BAND_ORDER.index(n) * 128 + 128]
    ps = s.ps("psD")
    _mm(nc, ps, B(bx), xplane, True, False)
    _mm(nc, ps, B(bg), gplane, False, False)
    _mm(nc, ps, B("BI"), addp, False, True)
    nc.scalar.copy(s.OSL(out_t, *ab), ps[:, 0:768])


def _hv_field(s, xa, xb, ga, gb, co, addp, out_t, ab, efix=None):
    """out = 0.5*((xa@co0+xb@co1) - 0.5*(ga@co0+gb@co1) + addp) at phase ab.
    efix: 'R'/'L' - the masked-plane sum t1 sees a zero replication neighbour at the
    right/left image edge column; compute that column from the surviving operand."""
    nc, V = s.nc, s.V
    t1 = s.tmp()
    if efix == "R":
        nc.vector.tensor_tensor(t1[:, 0:767], xa[:, 1 + co[0] : 768 + co[0]], xb[:, 1 + co[1] : 768 + co[1]], AL.add)
        nc.vector.tensor_copy(t1[:, 767:768], xa[:, 768 + co[0] : 769 + co[0]])
    elif efix == "L":
        nc.vector.tensor_tensor(t1[:, 1:768], xa[:, 2 + co[0] : 769 + co[0]], xb[:, 2 + co[1] : 769 + co[1]], AL.add)
        nc.vector.tensor_copy(t1[:, 0:1], xb[:, 1 + co[1] : 2 + co[1]])
    else:
        nc.vector.tensor_tensor(t1[:], V(xa, co[0]), V(xb, co[1]), AL.add)
    t2 = s.tmp(); nc.vector.tensor_tensor(t2[:], V(ga, co[0]), V(gb, co[1]), AL.add)
    u = s.tmp();  nc.vector.scalar_tensor_tensor(u[:], t2[:], -0.5, t1[:], AL.mult, AL.add)
    v = s.tmp();  nc.gpsimd.tensor_tensor(v[:], u[:], V(addp), AL.add)
    nc.scalar.mul(s.OSL(out_t, *ab), v[:], 0.5)


def _chan_blend(s, xp, xs, gf, gs, gnear, cP, cN, out_t, ab, efix=None):
    """R11/B00 diagonal interpolation.  xp: same-colour plane, xs: its vertical shift;
    gf: far green, gs: its shift; gnear: centre green.  cP/cN: column offsets
    (c_for_unshifted, c_for_shifted) of plus / minus diagonal pairs.  Comparison
    terms at half scale; result selected densely then interleaved into out."""
    nc, V = s.nc, s.V
    rp = s.tmp(); nc.vector.tensor_tensor(rp[:], V(xp, cP[0]), V(xs, cP[1]), AL.add)
    rn = s.tmp(); dn = s.tmp()
    if efix == "R":  # xp@cN0 hits zeroed right-edge replication at j=767
        nc.vector.tensor_tensor(rn[:, 0:767], xp[:, 1 + cN[0] : 768 + cN[0]], xs[:, 1 + cN[1] : 768 + cN[1]], AL.add)
        nc.vector.tensor_copy(rn[:, 767:768], xs[:, 768 + cN[1] : 769 + cN[1]])
        nc.gpsimd.tensor_tensor(dn[:, 0:767], xs[:, 1 + cN[1] : 768 + cN[1]], xp[:, 1 + cN[0] : 768 + cN[0]], AL.subtract)
        nc.vector.tensor_copy(dn[:, 767:768], xs[:, 768 + cN[1] : 769 + cN[1]])
    elif efix == "L":  # xs@cN1 hits zeroed left-edge replication at j=0
        nc.vector.tensor_tensor(rn[:, 1:768], xp[:, 2 + cN[0] : 769 + cN[0]], xs[:, 2 + cN[1] : 769 + cN[1]], AL.add)
        nc.vector.tensor_copy(rn[:, 0:1], xp[:, 1 + cN[0] : 2 + cN[0]])
        nc.gpsimd.tensor_tensor(dn[:, 1:768], xs[:, 2 + cN[1] : 769 + cN[1]], xp[:, 2 + cN[0] : 769 + cN[0]], AL.subtract)
        nc.vector.tensor_scalar(dn[:, 0:1], xp[:, 1 + cN[0] : 2 + cN[0]], -1.0, None, AL.mult)
    else:
        nc.vector.tensor_tensor(rn[:], V(xp, cN[0]), V(xs, cN[1]), AL.add)
        nc.gpsimd.tensor_tensor(dn[:], V(xs, cN[1]), V(xp, cN[0]), AL.subtract)
    dm = s.tmp(); nc.gpsimd.tensor_tensor(dm[:], V(xs, cP[1]), V(xp, cP[0]), AL.subtract)
    sp = s.tmp(); nc.gpsimd.tensor_tensor(sp[:], V(gf, cP[0]), V(gs, cP[1]), AL.add)
    sn = s.tmp(); nc.gpsimd.tensor_tensor(sn[:], V(gf, cN[0]), V(gs, cN[1]), AL.add)
    # chg2 = cg2/2, chg3 = cg3/2
    chg2 = s.tmp(); nc.vector.scalar_tensor_tensor(chg2[:], sp[:], 0.5, V(gnear), AL.mult, AL.subtract)
    chg3 = s.tmp(); nc.vector.scalar_tensor_tensor(chg3[:], sn[:], 0.5, V(gnear), AL.mult, AL.subtract)
    cp2 = s.tmp(); nc.vector.tensor_tensor(cp2[:], rp[:], chg2[:], AL.subtract)
    cn2 = s.tmp(); nc.vector.tensor_tensor(cn2[:], rn[:], chg3[:], AL.subtract)
    qp = s.tmp(); nc.scalar.activation(qp[:], chg2[:], AF.Abs)
    qn = s.tmp(); nc.scalar.activation(qn[:], chg3[:], AF.Abs)
    qdm = s.tmp(); nc.scalar.activation(qdm[:], dm[:], AF.Abs, scale=0.5)
    qdn = s.tmp(); nc.scalar.activation(qdn[:], dn[:], AF.Abs, scale=0.5)
    clp = s.tmp(); nc.vector.tensor_tensor(clp[:], qdm[:], qp[:], AL.add)
    cln = s.tmp(); nc.gpsimd.tensor_tensor(cln[:], qdn[:], qn[:], AL.add)
    dr = s.tmp(); nc.vector.tensor_tensor(dr[:], clp[:], cln[:], AL.subtract)
    mr = s.msk(); nc.vector.tensor_scalar(mr[:], dr[:], 0.0, None, AL.is_gt)
    res = s.tmp(); nc.vector.tensor_scalar(res[:], cp2[:], 0.5, None, AL.mult)
    cnh = s.tmp(); nc.scalar.mul(cnh[:], cn2[:], 0.5)
    nc.vector.copy_predicated(res[:], mr[:], cnh[:])
    nc.scalar.copy(s.OSL(out_t, *ab), res[:])


def _load_plane(nc, t, xc, lo, hi):
    """Cast-DMA plane rows [lo, hi) (clamped to [0,768)) so lane p = row lo+p."""
    clo, chi = max(lo, 0), min(hi, H)
    nc.gpsimd.dma_start(t[clo - lo : chi - lo, 1 : 1 + H], xc[clo:chi, :])


def _build_strip(nc, s, x, out_v, bands):
    r0, first, last = s.r0, s.first, s.last
    V = s.V
    base = r0 - 2  # lane p <-> row base+p

    for t, c in ((s.p00, 0), (s.p01, 1), (s.p10, 2), (s.p11, 3)):
        _load_plane(nc, t, x[c], base, base + 128)

    def fill(t, lane, c, row):
        nc.gpsimd.dma_start(t[lane : lane + 1, 1 : 1 + H], x[c, row : row + 1, :])

    # --- vertical edge replication on the MAIN tiles (virtual rows <0 / >767) ---
    if first:
        for t, c in ((s.p00, 0), (s.p01, 1), (s.p10, 0), (s.p11, 1)):
            fill(t, 0, c, 0); fill(t, 1, c, 0)
    if last:
        for t, c in ((s.p00, 2), (s.p01, 3), (s.p10, 2), (s.p11, 3)):
            fill(t, 126, c, 767); fill(t, 127, c, 767)

    # --- shifted tiles from SBUF shifts of main tiles (boundary lanes garbage,
    #     never shipped; image-edge lanes inherit the fills above) ---
    sshift = nc.sync.dma_start
    sshift(s.p00u[0:127, :], s.p00[1:128, :])
    sshift(s.p11d[1:128, :], s.p11[0:127, :])
    if first:
        fill(s.p11d, 0, 1, 0)  # lane 0 = virtual row base-1: replicate row 0
    if last:
        fill(s.p00u, 127, 2, 767)  # lane 127 = virtual row 768: replicate row 767

    # --- horizontal halo columns (mosaic col replication; cross-plane for odd cols) ---
    cc = nc.vector.tensor_copy
    cc(s.p00[:, 0:1], s.p00[:, 1:2])
    cc(s.p01[:, 0:1], s.p00[:, 1:2])
    cc(s.p11[:, 0:1], s.p10[:, 1:2])
    cc(s.p00[:, PW - 1 : PW], s.p01[:, PW - 2 : PW - 1])
    cc(s.p10[:, PW - 1 : PW], s.p11[:, PW - 2 : PW - 1])
    cc(s.p11[:, PW - 1 : PW], s.p11[:, PW - 2 : PW - 1])
    nc.vector.memset(s.p00u[:, PW - 1 : PW], 0.0)  # xc_r replication at right edge is 0
    nc.vector.memset(s.p11d[:, 0:1], 0.0)          # xc_b replication at left edge is 0

    # red/blue vertical shifts need zero (masked) replication at image top/bottom:
    # use zero-lane variants of p00u (last strip) / p11d (first strip)
    p00u_rb, p11d_rb = s.p00u, s.p11d
    if s.last:
        p00u_rb = s.p00uz
        nc.vector.memset(p00u_rb[:], 0.0)
        nc.sync.dma_start(p00u_rb[0:125, :], s.p00[1:126, :])
    if s.first:
        p11d_rb = s.p11dz
        nc.vector.memset(p11d_rb[:], 0.0)
        nc.sync.dma_start(p11d_rb[3:128, :], s.p11[2:127, :])

    # --- green interpolation (vertical parts on the tensor engine) ---
    # phase 00: centre p00, horiz p01 (j-1, j), vert plane p10 (taps i-1, i)
    _green_phase(s, s.p00, s.p01, (-1, 0), s.p10, bands, "BA", "BC5", s.g00)
    # phase 11: centre p11, horiz p10 (j, j+1), vert plane p01 (taps i, i+1)
    _green_phase(s, s.p11, s.p10, (0, +1), s.p01, bands, "BA2", "BC5b", s.g11)

    cc(s.g00[:, PW - 1 : PW], s.p01[:, PW - 2 : PW - 1])  # G00[:,768] = P01[:,767]
    cc(s.g11[:, 0:1], s.p10[:, 1:2])                      # G11[:,-1] = P10[:,0]
    if first:  # green at virtual row -1 (lane 1): g11 = P01[0] (= p01 lane 1)
        nc.sync.dma_start(s.g11[1:2, :], s.p01[1:2, :])
    if last:   # green at virtual row 768 (lane 126): g00 = P10[767] (= p10 lane 126)
        nc.sync.dma_start(s.g00[126:127, :], s.p10[126:127, :])

    # shifted green tiles (SBUF->SBUF row shift)
    nc.sync.dma_start(s.g00u[0:127, :], s.g00[1:128, :])
    nc.sync.dma_start(s.g11d[1:128, :], s.g11[0:127, :])

    # --- green output ---
    nc.vector.tensor_copy(s.OSL(s.out_g, 0, 0), V(s.g00))
    nc.vector.tensor_copy(s.OSL(s.out_g, 1, 1), V(s.g11))
    nc.scalar.copy(s.OSL(s.out_g, 0, 1), V(s.p01))
    nc.scalar.copy(s.OSL(s.out_g, 1, 0), V(s.p10))

    # --- red ---
    nc.vector.tensor_copy(s.OSL(s.out_r, 0, 0), V(s.p00))
    _hv_field(s, s.p00, s.p00, s.g00, s.g00, (0, +1), s.p01, s.out_r, (0, 1), efix="R")
    if last:  # needs the zero-replication variant: legacy vector path
        _hv_field(s, s.p00, p00u_rb, s.g00, s.g00u, (0, 0), s.p10, s.out_r, (1, 0))
    else:     # R10 = 0.5*(P00[i]+P00[i+1]) - 0.25*(G00[i]+G00[i+1]) + 0.5*P10
        _hv_vert_banded(s, bands, "BA2", s.p00, "BE", s.g00, s.p10, s.out_r, (1, 0))
    # R11: P pair (0,0)+(+1,+1), N pair (0,+1)+(+1,0); shifted operand = (+1,*)
    _chan_blend(s, s.p00, p00u_rb, s.g00, s.g00u, s.g11, (0, +1), (+1, 0), s.out_r, (1, 1), efix="R")

    # --- blue ---
    nc.vector.tensor_copy(s.OSL(s.out_b, 1, 1), V(s.p11))
    _hv_field(s, s.p11, s.p11, s.g11, s.g11, (-1, 0), s.p10, s.out_b, (1, 0), efix="L")
    if first:  # needs the zero-replication variant: legacy vector path
        _hv_field(s, p11d_rb, s.p11, s.g11d, s.g11, (0, 0), s.p01, s.out_b, (0, 1))
    else:      # B01 = 0.5*(P11[i-1]+P11[i]) - 0.25*(G11[i-1]+G11[i]) + 0.5*P01
        _hv_vert_banded(s, bands, "BA", s.p11, "BF", s.g11, s.p01, s.out_b, (0, 1))
    # B00: P pair p11d@(-1)+p11@(0); N pair p11d@(0)+p11@(-1)
    _chan_blend(s, p11d_rb, s.p11, s.g11d, s.g11, s.g00, (-1, 0), (0, -1), s.out_b, (0, 0), efix="L")

    # --- output DMA (lanes 2..125 <-> rows r0..r0+123; last strip ships 102..125) ---
    if last:
        p0, pn, row0 = 102, 24, 744
    else:
        p0, pn, row0 = 2, 124, r0
    for c, t in enumerate((s.out_r, s.out_g, s.out_b)):
        nc.gpsimd.dma_start(out_v[c, row0 : row0 + pn, :], t[p0 : p0 + pn, :])


def build_nc():
    nc = bacc.Bacc("TRN2", target_bir_lowering=False, debug=False, num_devices=NCORES)
    x_in = nc.declare_dram_parameter("x", [4, H, H], F, isOutput=False)
    bands_in = nc.declare_dram_parameter("bands", [128, len(BAND_ORDER) * 128], H16, isOutput=False)
    out = nc.declare_dram_parameter("out", [3, 2 * H, 2 * H], F, isOutput=True)
    out_v = out[:].rearrange("c (r two) w -> c r (two w)", two=2)

    with tile.TileContext(nc) as tc, ExitStack() as ctx:
        cst = ctx.enter_context(tc.tile_pool(name="consts", bufs=1))
        pl = ctx.enter_context(tc.tile_pool(name="planes", bufs=3))
        gr = ctx.enter_context(tc.tile_pool(name="greens", bufs=3))
        tmp = ctx.enter_context(tc.tile_pool(name="temps", bufs=16))
        outp = ctx.enter_context(tc.tile_pool(name="outs", bufs=4))
        psp = ctx.enter_context(tc.tile_pool(name="ps", bufs=1, space=bass.MemorySpace.PSUM))
        bands = cst.tile([128, len(BAND_ORDER) * 128], H16, tag="bands", name="bands")
        nc.sync.dma_start(bands[:], bands_in[:])
        for si, r0 in enumerate(STRIPS):
            s = S(nc, (pl, gr, tmp, outp, psp), r0, si == 0, si == len(STRIPS) - 1)
            _build_strip(nc, s, x_in[:], out_v, bands)
    nc.compile()
    return nc


_NC_CACHE = None


def kernel(x: np.ndarray) -> np.ndarray:
    global _NC_CACHE
    if _NC_CACHE is None:
        _NC_CACHE = build_nc()
    x = np.ascontiguousarray(x, dtype=np.float32)
    bnp = _bands_np()
    in_maps = [{"x": x[i], "bands": bnp} for i in range(NCORES)]
    res = run_bass_kernel_spmd(_NC_CACHE, in_maps, list(range(NCORES)))
    return np.stack([res.results[i]["out"] for i in range(NCORES)], axis=0)

